# revision 5
# baseline (speedup 1.0000x reference)
"""Trainium2 Bass kernel for nn_DiagonalTransfer.

Computes out[i, m] = logsumexp_j( D[i, j] + xx[j, m] ) with D = diag(diag),
using the exact algebraic reduction (off-diagonal zeros contribute exp(xx)):

    out[i, m] = log( S_m + c_i * exp(xx[i, m]) ),   S_m = sum_j exp(xx[j, m]),
    c_i = expm1(diag_i)

Distribution: xx is transposed on the host and sharded row-wise over 8 cores,
so each core holds a contiguous (512, 4096) block with columns m of xx on the
SBUF partition axis and the reduction axis j on the free axis. Per [128, 4096]
tile the whole computation is two ScalarE activations:

    Exp with accum_out  -> E = exp(x), S = free-dim sum (per-partition scalar)
    Ln(scale*E + bias)  -> out, scale = expm1(diag) (constant when diag == 1),
                           bias = S (per-partition AP)

No cross-core communication; the column-wise sums are local to a shard.

Sync note: each DMA gets a semaphore with at most one transfer in flight —
with two DMAs pending on one sem, the 16 per-engine increments of both
transfers interleave, so a wait for >= 16 can fire before either transfer
fully landed (caught by the CoreSim race detector).
"""

import numpy as np

N = 4096  # rows of xx == reduction axis j == index of diag
M = 4096  # columns of xx == sharded axis m
NCORES = 8
MS = M // NCORES  # 512 columns per core
P = 128           # SBUF partitions
NT = MS // P      # 4 tiles of [128, N] per core
NB = 2            # double buffering

_program_cache = {}


def _build(scale_const):
    """Build the per-core Bass program.

    scale_const: float -> all-ones diag fast path (scale immediate in the Ln
    activation). None -> general path: an extra input "cb" of shape [P, N]
    holds expm1(diag) broadcast across partitions; a DVE multiply applies it.
    """
    import concourse.bass as bass
    import concourse.mybir as mybir

    dt = mybir.dt.float32
    AF = mybir.ActivationFunctionType

    nc = bass.Bass()
    x = nc.declare_dram_parameter("x", [MS, N], dt, isOutput=False)
    general = scale_const is None
    if general:
        cb = nc.declare_dram_parameter("cb", [P, N], dt, isOutput=False)
    out = nc.declare_dram_parameter("out", [MS, N], dt, isOutput=True)

    from contextlib import ExitStack

    with ExitStack() as ctx:
        X = ctx.enter_context(nc.sbuf_tensor([P, NB * N], dt))
        E = ctx.enter_context(nc.sbuf_tensor([P, NB * N], dt))
        O = ctx.enter_context(nc.sbuf_tensor([P, NB * N], dt))
        S = ctx.enter_context(nc.sbuf_tensor([P, NB], dt))
        if general:
            CB = ctx.enter_context(nc.sbuf_tensor([P, N], dt))
            EC = ctx.enter_context(nc.sbuf_tensor([P, NB * N], dt))
        # One DMA in flight per semaphore: a slot's loads (stores) are
        # serialized by the compute dependency, so per-slot sems suffice.
        in_s = [
            ctx.enter_context(nc.semaphore(name=f"in_s{b}")) for b in range(NB)
        ]
        out_s = [
            ctx.enter_context(nc.semaphore(name=f"out_s{b}")) for b in range(NB)
        ]
        exp_sem = ctx.enter_context(nc.semaphore(name="exp_sem"))
        ln_sem = ctx.enter_context(nc.semaphore(name="ln_sem"))
        if general:
            cb_sem = ctx.enter_context(nc.semaphore(name="cb_sem"))
            mul_sem = ctx.enter_context(nc.semaphore(name="mul_sem"))
        block = ctx.enter_context(nc.Block())

        @block.sync
        def _(sync):
            if general:
                sync.dma_start(out=CB[:], in_=cb[:]).then_inc(cb_sem, 16)
            for t in range(NT):
                b, k = t % NB, t // NB
                if t >= NB:
                    # X[b] is free once exp of tile t-NB has consumed it.
                    sync.wait_ge(exp_sem, t - NB + 1)
                sync.dma_start(
                    out=X[:, b * N:(b + 1) * N], in_=x[t * P:(t + 1) * P, :]
                ).then_inc(in_s[b], 16)
            # all stores landed before kernel exit
            for b in range(NB):
                sync.wait_ge(out_s[b], (NT // NB) * 16)

        if general:

            @block.vector
            def _(vector):
                vector.wait_ge(cb_sem, 16)  # CB resident
                for t in range(NT):
                    b = t % NB
                    vector.wait_ge(exp_sem, t + 1)  # E[b] written
                    if t >= NB:
                        # EC[b] free once Ln of tile t-NB has read it.
                        vector.wait_ge(ln_sem, t - NB + 1)
                    vector.tensor_mul(
                        EC[:, b * N:(b + 1) * N], E[:, b * N:(b + 1) * N], CB[:]
                    ).then_inc(mul_sem, 1)

        @block.scalar
        def _(scalar):
            for t in range(NT):
                b, k = t % NB, t // NB
                scalar.wait_ge(in_s[b], (k + 1) * 16)  # X[b] loaded
                if t >= NB:
                    # E[b]/S[b] free once their tile t-NB readers are done.
                    scalar.wait_ge(ln_sem, t - NB + 1)
                    if general:
                        scalar.wait_ge(mul_sem, t - NB + 1)
                scalar.activation(
                    E[:, b * N:(b + 1) * N],
                    X[:, b * N:(b + 1) * N],
                    AF.Exp,
                    accum_out=S[:, b:b + 1],
                ).then_inc(exp_sem, 1)
                # Same-engine wait: ACT pipelines back-to-back activations, so
                # Ln's read of E/S needs an explicit sem wait on exp completion.
                scalar.wait_ge(exp_sem, t + 1)
                if t >= NB:
                    # O[b] free once the store of tile t-NB has landed.
                    scalar.wait_ge(out_s[b], k * 16)
                if general:
                    scalar.wait_ge(mul_sem, t + 1)  # EC[b] ready
                    src = EC
                    scale = 1.0
                else:
                    src = E
                    scale = scale_const
                scalar.activation(
                    O[:, b * N:(b + 1) * N],
                    src[:, b * N:(b + 1) * N],
                    AF.Ln,
                    bias=S[:, b:b + 1],
                    scale=scale,
                ).then_inc(ln_sem, 1)
                # O[b] visible before the store trigger (same-engine DMA read).
                scalar.wait_ge(ln_sem, t + 1)
                scalar.dma_start(
                    out=out[t * P:(t + 1) * P, :], in_=O[:, b * N:(b + 1) * N]
                ).then_inc(out_s[b], 16)

    return nc


def _run(xx, diag, trace=False):
    from concourse.bass_utils import run_bass_kernel_spmd

    xx = np.asarray(xx, dtype=np.float32)
    diag = np.asarray(diag, dtype=np.float32)
    assert xx.shape == (N, M) and diag.shape == (N,)

    ones = bool(np.all(diag == 1.0))
    key = "ones" if ones else "general"
    if key not in _program_cache:
        _program_cache[key] = _build(float(np.expm1(1.0)) if ones else None)
    nc = _program_cache[key]

    xxT = np.ascontiguousarray(xx.T)  # (M, N): row m holds all j of column m
    in_maps = []
    for c in range(NCORES):
        m = {"x": xxT[c * MS:(c + 1) * MS]}
        if not ones:
            cbv = np.ascontiguousarray(
                np.broadcast_to(np.expm1(diag)[None, :], (P, N))
            ).astype(np.float32)
            m["cb"] = cbv
        in_maps.append(m)

    res = run_bass_kernel_spmd(nc, in_maps, core_ids=list(range(NCORES)), trace=trace)
    outT = np.concatenate([r["out"] for r in res.results], axis=0)  # (M, N)
    return np.ascontiguousarray(outT.T), res


def kernel(xx, diag):
    out, _ = _run(xx, diag)
    return out


# revision 8
# speedup vs baseline: 1.2550x; 1.2550x over previous
"""Trainium2 Bass kernel for nn_DiagonalTransfer.

Computes out[i, m] = logsumexp_j( D[i, j] + xx[j, m] ) with D = diag(diag),
using the exact algebraic reduction (off-diagonal zeros contribute exp(xx)):

    out[i, m] = log( S_m + c_i * exp(xx[i, m]) ),   S_m = sum_j exp(xx[j, m]),
    c_i = expm1(diag_i)

Distribution: xx is transposed on the host and sharded row-wise over 8 cores,
so each core holds a contiguous (512, 4096) block with columns m of xx on the
SBUF partition axis and the reduction axis j on the free axis. Per [128, 4096]
tile the computation is two ScalarE activation passes:

    Exp with accum_out  -> E = exp(x), S = free-dim sum (per-partition scalar)
    Ln(scale*E + bias)  -> out, scale = expm1(diag) (constant when diag == 1),
                           bias = S (per-partition AP)

The fast (diag == 1) path pipelines at 1 MB chunk granularity (2 chunks per
tile): loads, Exp, Ln and stores all stream, partial Exp accumulators are
combined by a tiny DVE reduce, and all DMA triggers run on the Sync engine so
the Scalar engine only executes back-to-back activations.

No cross-core communication; the column-wise sums are local to a shard.

Sync note: every DMA gets a semaphore with at most one transfer in flight —
with two DMAs pending on one sem, the 16 per-engine increments of both
transfers interleave, so a wait for >= 16 can fire before either transfer
fully landed (caught by the CoreSim race detector). ACT also pipelines
back-to-back activations, so same-engine RAW deps need explicit sem waits.
"""

import numpy as np

N = 4096  # rows of xx == reduction axis j == index of diag
M = 4096  # columns of xx == sharded axis m
NCORES = 8
MS = M // NCORES  # 512 columns per core
P = 128           # SBUF partitions
NT = MS // P      # 4 tiles of [128, N] per core
CH = 2            # chunks per tile (1 MB DMA granularity)
FDC = N // CH     # free-dim elements per chunk
NBX = 2           # X/E slot double buffering
NBO = 3           # O slots

_program_cache = {}


def _build_fast(scale_const):
    """diag == 1 path: chunked streaming pipeline, DMA triggers on Sync."""
    import concourse.bass as bass
    import concourse.mybir as mybir

    dt = mybir.dt.float32
    AF = mybir.ActivationFunctionType

    nc = bass.Bass()
    x = nc.declare_dram_parameter("x", [MS, N], dt, isOutput=False)
    out = nc.declare_dram_parameter("out", [MS, N], dt, isOutput=True)

    from contextlib import ExitStack

    with ExitStack() as ctx:
        X = ctx.enter_context(nc.sbuf_tensor([P, NBX * N], dt))
        E = ctx.enter_context(nc.sbuf_tensor([P, NBX * N], dt))
        O = ctx.enter_context(nc.sbuf_tensor([P, NBO * N], dt))
        S2 = ctx.enter_context(nc.sbuf_tensor([P, NT * CH], dt))  # per-chunk sums
        Sc = ctx.enter_context(nc.sbuf_tensor([P, NT], dt))       # per-tile sums
        D = ctx.enter_context(nc.sbuf_tensor([P, 2], dt))         # table-warm scratch

        in_s = [
            [ctx.enter_context(nc.semaphore(name=f"in_s{b}_{c}")) for c in range(CH)]
            for b in range(NBX)
        ]
        out_s = [
            [ctx.enter_context(nc.semaphore(name=f"out_s{b}_{c}")) for c in range(CH)]
            for b in range(NBO)
        ]
        exp_sem = ctx.enter_context(nc.semaphore(name="exp_sem"))
        cmb_sem = ctx.enter_context(nc.semaphore(name="cmb_sem"))
        ln_sem = ctx.enter_context(nc.semaphore(name="ln_sem"))
        block = ctx.enter_context(nc.Block())

        def xsl(t, c):
            return X[:, (t % NBX) * N + c * FDC:(t % NBX) * N + (c + 1) * FDC]

        def esl(t, c):
            return E[:, (t % NBX) * N + c * FDC:(t % NBX) * N + (c + 1) * FDC]

        def osl(t, c):
            return O[:, (t % NBO) * N + c * FDC:(t % NBO) * N + (c + 1) * FDC]

        def dram_sl(tensor, t, c):
            return tensor[t * P:(t + 1) * P, c * FDC:(c + 1) * FDC]

        @block.sync
        def _(sync):
            # SP executes in order: loads as early as slot reuse allows,
            # stores interleaved as their Ln completes.
            order = [
                ("L", 0, 0), ("L", 0, 1), ("L", 1, 0), ("L", 1, 1),
                ("L", 2, 0), ("L", 2, 1), ("S", 0, 0), ("L", 3, 0),
                ("S", 0, 1), ("L", 3, 1), ("S", 1, 0), ("S", 1, 1),
                ("S", 2, 0), ("S", 2, 1), ("S", 3, 0), ("S", 3, 1),
            ]
            for kind, t, c in order:
                if kind == "L":
                    if t >= NBX:
                        # X slot chunk free once exp of tile t-NBX consumed it
                        sync.wait_ge(exp_sem, (t - NBX) * CH + c + 1)
                    sync.dma_start(
                        out=xsl(t, c), in_=dram_sl(x, t, c)
                    ).then_inc(in_s[t % NBX][c], 16)
                else:
                    sync.wait_ge(ln_sem, t * CH + c + 1)
                    sync.dma_start(
                        out=dram_sl(out, t, c), in_=osl(t, c)
                    ).then_inc(out_s[t % NBO][c], 16)
            for b in range(NBO):
                n_tiles_in_slot = len([t for t in range(NT) if t % NBO == b])
                for c in range(CH):
                    sync.wait_ge(out_s[b][c], n_tiles_in_slot * 16)

        @block.vector
        def _(vector):
            for t in range(NT):
                vector.wait_ge(exp_sem, t * CH + CH)  # both chunk sums written
                vector.reduce_sum(
                    Sc[:, t:t + 1], S2[:, t * CH:(t + 1) * CH],
                    axis=mybir.AxisListType.X,
                ).then_inc(cmb_sem, 1)

        @block.scalar
        def _(scalar):
            # warm the exp/ln table set while the first load is in flight
            zero_ap = nc.const_aps.aps[(dt, 0.0)]
            scalar.activation(D[:, 0:1], zero_ap, AF.Exp)

            def emit_exp(t, c):
                b = t % NBX
                scalar.wait_ge(in_s[b][c], (t // NBX + 1) * 16)
                if t >= NBX:
                    # E slot chunk free once Ln of tile t-NBX read it
                    scalar.wait_ge(ln_sem, (t - NBX) * CH + c + 1)
                scalar.activation(
                    esl(t, c), xsl(t, c), AF.Exp,
                    accum_out=S2[:, t * CH + c:t * CH + c + 1],
                ).then_inc(exp_sem, 1)

            def emit_ln(t, c):
                scalar.wait_ge(cmb_sem, t + 1)  # Sc ready (covers exp via DVE)
                if t >= NBO:
                    # O slot chunk free once store of tile t-NBO landed
                    scalar.wait_ge(out_s[t % NBO][c], (t // NBO) * 16)
                scalar.activation(
                    osl(t, c), esl(t, c), AF.Ln,
                    bias=Sc[:, t:t + 1], scale=scale_const,
                ).then_inc(ln_sem, 1)

            # e00 e01 | e(t,0) l(t-1,0) e(t,1) l(t-1,1) | l30 l31
            emit_exp(0, 0)
            emit_exp(0, 1)
            for t in range(1, NT):
                emit_exp(t, 0)
                emit_ln(t - 1, 0)
                emit_exp(t, 1)
                emit_ln(t - 1, 1)
            emit_ln(NT - 1, 0)
            emit_ln(NT - 1, 1)

    return nc


def _build_general():
    """General diag path: unchunked double-buffered pipeline with a DVE
    multiply by cb = expm1(diag) broadcast across partitions."""
    import concourse.bass as bass
    import concourse.mybir as mybir

    dt = mybir.dt.float32
    AF = mybir.ActivationFunctionType
    NB = 2

    nc = bass.Bass()
    x = nc.declare_dram_parameter("x", [MS, N], dt, isOutput=False)
    cb = nc.declare_dram_parameter("cb", [P, N], dt, isOutput=False)
    out = nc.declare_dram_parameter("out", [MS, N], dt, isOutput=True)

    from contextlib import ExitStack

    with ExitStack() as ctx:
        X = ctx.enter_context(nc.sbuf_tensor([P, NB * N], dt))
        E = ctx.enter_context(nc.sbuf_tensor([P, NB * N], dt))
        O = ctx.enter_context(nc.sbuf_tensor([P, NB * N], dt))
        S = ctx.enter_context(nc.sbuf_tensor([P, NB], dt))
        CB = ctx.enter_context(nc.sbuf_tensor([P, N], dt))
        EC = ctx.enter_context(nc.sbuf_tensor([P, NB * N], dt))
        in_s = [
            ctx.enter_context(nc.semaphore(name=f"in_s{b}")) for b in range(NB)
        ]
        out_s = [
            ctx.enter_context(nc.semaphore(name=f"out_s{b}")) for b in range(NB)
        ]
        exp_sem = ctx.enter_context(nc.semaphore(name="exp_sem"))
        ln_sem = ctx.enter_context(nc.semaphore(name="ln_sem"))
        cb_sem = ctx.enter_context(nc.semaphore(name="cb_sem"))
        mul_sem = ctx.enter_context(nc.semaphore(name="mul_sem"))
        block = ctx.enter_context(nc.Block())

        @block.sync
        def _(sync):
            sync.dma_start(out=CB[:], in_=cb[:]).then_inc(cb_sem, 16)
            for t in range(NT):
                b = t % NB
                if t >= NB:
                    sync.wait_ge(exp_sem, t - NB + 1)
                sync.dma_start(
                    out=X[:, b * N:(b + 1) * N], in_=x[t * P:(t + 1) * P, :]
                ).then_inc(in_s[b], 16)
            for b in range(NB):
                sync.wait_ge(out_s[b], (NT // NB) * 16)

        @block.vector
        def _(vector):
            vector.wait_ge(cb_sem, 16)
            for t in range(NT):
                b = t % NB
                vector.wait_ge(exp_sem, t + 1)
                if t >= NB:
                    vector.wait_ge(ln_sem, t - NB + 1)
                vector.tensor_mul(
                    EC[:, b * N:(b + 1) * N], E[:, b * N:(b + 1) * N], CB[:]
                ).then_inc(mul_sem, 1)

        @block.scalar
        def _(scalar):
            for t in range(NT):
                b, k = t % NB, t // NB
                scalar.wait_ge(in_s[b], (k + 1) * 16)
                if t >= NB:
                    scalar.wait_ge(mul_sem, t - NB + 1)
                scalar.activation(
                    E[:, b * N:(b + 1) * N],
                    X[:, b * N:(b + 1) * N],
                    AF.Exp,
                    accum_out=S[:, b:b + 1],
                ).then_inc(exp_sem, 1)
                scalar.wait_ge(mul_sem, t + 1)
                if t >= NB:
                    scalar.wait_ge(out_s[b], k * 16)
                scalar.activation(
                    O[:, b * N:(b + 1) * N],
                    EC[:, b * N:(b + 1) * N],
                    AF.Ln,
                    bias=S[:, b:b + 1],
                    scale=1.0,
                ).then_inc(ln_sem, 1)
                scalar.wait_ge(ln_sem, t + 1)
                scalar.dma_start(
                    out=out[t * P:(t + 1) * P, :], in_=O[:, b * N:(b + 1) * N]
                ).then_inc(out_s[b], 16)

    return nc


def _run(xx, diag, trace=False):
    from concourse.bass_utils import run_bass_kernel_spmd

    xx = np.asarray(xx, dtype=np.float32)
    diag = np.asarray(diag, dtype=np.float32)
    assert xx.shape == (N, M) and diag.shape == (N,)

    ones = bool(np.all(diag == 1.0))
    key = "ones" if ones else "general"
    if key not in _program_cache:
        _program_cache[key] = (
            _build_fast(float(np.expm1(1.0))) if ones else _build_general()
        )
    nc = _program_cache[key]

    xxT = np.ascontiguousarray(xx.T)  # (M, N): row m holds all j of column m
    in_maps = []
    for c in range(NCORES):
        m = {"x": xxT[c * MS:(c + 1) * MS]}
        if not ones:
            cbv = np.ascontiguousarray(
                np.broadcast_to(np.expm1(diag)[None, :], (P, N))
            ).astype(np.float32)
            m["cb"] = cbv
        in_maps.append(m)

    res = run_bass_kernel_spmd(nc, in_maps, core_ids=list(range(NCORES)), trace=trace)
    outT = np.concatenate([r["out"] for r in res.results], axis=0)  # (M, N)
    return np.ascontiguousarray(outT.T), res


def kernel(xx, diag):
    out, _ = _run(xx, diag)
    return out


# revision 13
# speedup vs baseline: 1.2667x; 1.0093x over previous
"""Trainium2 Bass kernel for nn_DiagonalTransfer.

Computes out[i, m] = logsumexp_j( D[i, j] + xx[j, m] ) with D = diag(diag),
using the exact algebraic reduction (off-diagonal zeros contribute exp(xx)):

    out[i, m] = log( S_m + c_i * exp(xx[i, m]) ),   S_m = sum_j exp(xx[j, m]),
    c_i = expm1(diag_i)

Distribution: xx is transposed on the host and sharded row-wise over 8 cores,
so each core holds a contiguous (512, 4096) block with columns m of xx on the
SBUF partition axis and the reduction axis j on the free axis. Per [128, 4096]
tile the computation is two ScalarE activation passes:

    Exp with accum_out  -> E = exp(x), S = free-dim sum (per-partition scalar)
    Ln(scale*E + bias)  -> out, scale = expm1(diag) (constant when diag == 1),
                           bias = S (per-partition AP)

The fast (diag == 1) path pipelines at 1 MB chunk granularity (2 chunks per
tile): loads, Exp, Ln and stores all stream, partial Exp accumulators are
combined by a tiny DVE reduce, and all DMA triggers run on the Sync engine so
the Scalar engine only executes back-to-back activations.

No cross-core communication; the column-wise sums are local to a shard.

Sync note: every DMA gets a semaphore with at most one transfer in flight —
with two DMAs pending on one sem, the 16 per-engine increments of both
transfers interleave, so a wait for >= 16 can fire before either transfer
fully landed (caught by the CoreSim race detector). ACT also pipelines
back-to-back activations, so same-engine RAW deps need explicit sem waits.
"""

import numpy as np

N = 4096  # rows of xx == reduction axis j == index of diag
M = 4096  # columns of xx == sharded axis m
NCORES = 8
MS = M // NCORES  # 512 columns per core
P = 128           # SBUF partitions
NT = MS // P      # 4 tiles of [128, N] per core
CH = 2            # chunks per tile (1 MB DMA granularity)
FDC = N // CH     # free-dim elements per chunk
NBX = 3           # X/E slots
NBO = 3           # O slots
# Ln/store chunk widths per tile (elements); the last tile tapers so the
# final store is small and the drain tail is short.
LN_W = {0: [2048, 2048], 1: [2048, 2048], 2: [2048, 2048], 3: [2048, 1024, 1024]}

_program_cache = {}


def _build_fast(scale_const):
    """diag == 1 path: chunked streaming pipeline, DMA triggers on Sync."""
    import concourse.bass as bass
    import concourse.mybir as mybir

    dt = mybir.dt.float32
    AF = mybir.ActivationFunctionType

    nc = bass.Bass()
    x = nc.declare_dram_parameter("x", [MS, N], dt, isOutput=False)
    out = nc.declare_dram_parameter("out", [MS, N], dt, isOutput=True)

    from contextlib import ExitStack

    # Ln/store chunk element offsets per tile from LN_W widths
    ln_off = {}
    for t, ws in LN_W.items():
        offs, acc = [], 0
        for w in ws:
            offs.append((acc, w))
            acc += w
        assert acc == N
        ln_off[t] = offs
    # global 1-based ln index per (tile, chunk), in ACT emission order
    ln_idx = {}
    gi = 0
    for t in range(NT):
        for ci in range(len(LN_W[t])):
            gi += 1
            ln_idx[(t, ci)] = gi
    ln_total = {t: ln_idx[(t, len(LN_W[t]) - 1)] for t in range(NT)}

    with ExitStack() as ctx:
        X = ctx.enter_context(nc.sbuf_tensor([P, NBX * N], dt))
        E = ctx.enter_context(nc.sbuf_tensor([P, NBX * N], dt))
        O = ctx.enter_context(nc.sbuf_tensor([P, NBO * N], dt))
        S2 = ctx.enter_context(nc.sbuf_tensor([P, NT * CH], dt))  # per-chunk sums
        Sc = ctx.enter_context(nc.sbuf_tensor([P, NT], dt))       # per-tile sums
        D = ctx.enter_context(nc.sbuf_tensor([P, 2], dt))         # table-warm scratch

        max_ln_ch = max(len(w) for w in LN_W.values())
        in_s = [
            [ctx.enter_context(nc.semaphore(name=f"in_s{b}_{c}")) for c in range(CH)]
            for b in range(NBX)
        ]
        out_s = [
            [
                ctx.enter_context(nc.semaphore(name=f"out_s{b}_{c}"))
                for c in range(max_ln_ch)
            ]
            for b in range(NBO)
        ]
        exp_sem = ctx.enter_context(nc.semaphore(name="exp_sem"))
        cmb_sem = ctx.enter_context(nc.semaphore(name="cmb_sem"))
        ln_sem = ctx.enter_context(nc.semaphore(name="ln_sem"))
        block = ctx.enter_context(nc.Block())

        def xsl(t, c):
            return X[:, (t % NBX) * N + c * FDC:(t % NBX) * N + (c + 1) * FDC]

        def esl(t, c):
            return E[:, (t % NBX) * N + c * FDC:(t % NBX) * N + (c + 1) * FDC]

        def e_ln_sl(t, ci):
            off, w = ln_off[t][ci]
            return E[:, (t % NBX) * N + off:(t % NBX) * N + off + w]

        def o_ln_sl(t, ci):
            off, w = ln_off[t][ci]
            return O[:, (t % NBO) * N + off:(t % NBO) * N + off + w]

        def dram_sl_chunk(tensor, t, c):
            return tensor[t * P:(t + 1) * P, c * FDC:(c + 1) * FDC]

        # count of stores per (slot, pos) in program order, for wait values
        store_occ = {}
        store_wait_val = {}
        for t in range(NT):
            for ci in range(len(LN_W[t])):
                k = (t % NBO, ci)
                store_occ[k] = store_occ.get(k, 0) + 1
                store_wait_val[(t, ci)] = store_occ[k] * 16

        @block.sync
        def _(sync):
            # all loads up front (X is triple-buffered so only tile 3 waits),
            # stores follow as their Ln completes; SP executes in order.
            for t in range(NT):
                for c in range(CH):
                    if t >= NBX:
                        # X slot free once every exp of tile t-NBX consumed it
                        sync.wait_ge(exp_sem, (t - NBX) * CH + CH)
                    sync.dma_start(
                        out=xsl(t, c), in_=dram_sl_chunk(x, t, c)
                    ).then_inc(in_s[t % NBX][c], 16)
            for t in range(NT):
                for ci in range(len(LN_W[t])):
                    sync.wait_ge(ln_sem, ln_idx[(t, ci)])
                    off, w = ln_off[t][ci]
                    sync.dma_start(
                        out=out[t * P:(t + 1) * P, off:off + w],
                        in_=o_ln_sl(t, ci),
                    ).then_inc(out_s[t % NBO][ci], 16)
            for (b, ci), occ in sorted(store_occ.items()):
                sync.wait_ge(out_s[b][ci], occ * 16)

        @block.vector
        def _(vector):
            for t in range(NT):
                vector.wait_ge(exp_sem, t * CH + CH)  # both chunk sums written
                vector.reduce_sum(
                    Sc[:, t:t + 1], S2[:, t * CH:(t + 1) * CH],
                    axis=mybir.AxisListType.X,
                ).then_inc(cmb_sem, 1)

        @block.scalar
        def _(scalar):
            # warm the exp/ln table set while the first load is in flight
            zero_ap = nc.const_aps.aps[(dt, 0.0)]
            scalar.activation(D[:, 0:1], zero_ap, AF.Exp)

            def emit_exp(t, c):
                b = t % NBX
                scalar.wait_ge(in_s[b][c], (t // NBX + 1) * 16)
                if t >= NBX:
                    # E slot free once every Ln of tile t-NBX read it
                    scalar.wait_ge(ln_sem, ln_total[t - NBX])
                scalar.activation(
                    esl(t, c), xsl(t, c), AF.Exp,
                    accum_out=S2[:, t * CH + c:t * CH + c + 1],
                ).then_inc(exp_sem, 1)

            def emit_ln(t, ci):
                scalar.wait_ge(cmb_sem, t + 1)  # Sc ready (covers exp via DVE)
                if t >= NBO:
                    # O slot free once every store of tile t-NBO landed
                    for cj in range(len(LN_W[t - NBO])):
                        scalar.wait_ge(
                            out_s[(t - NBO) % NBO][cj],
                            store_wait_val[(t - NBO, cj)],
                        )
                scalar.activation(
                    o_ln_sl(t, ci), e_ln_sl(t, ci), AF.Ln,
                    bias=Sc[:, t:t + 1], scale=scale_const,
                ).then_inc(ln_sem, 1)

            # e00 e01 | e(t,0) l(t-1,0) e(t,1) l(t-1,1) | lns of last tile
            emit_exp(0, 0)
            emit_exp(0, 1)
            for t in range(1, NT):
                emit_exp(t, 0)
                emit_ln(t - 1, 0)
                emit_exp(t, 1)
                emit_ln(t - 1, 1)
            for ci in range(len(LN_W[NT - 1])):
                emit_ln(NT - 1, ci)

    return nc


def _build_general():
    """General diag path: unchunked double-buffered pipeline with a DVE
    multiply by cb = expm1(diag) broadcast across partitions."""
    import concourse.bass as bass
    import concourse.mybir as mybir

    dt = mybir.dt.float32
    AF = mybir.ActivationFunctionType
    NB = 2

    nc = bass.Bass()
    x = nc.declare_dram_parameter("x", [MS, N], dt, isOutput=False)
    cb = nc.declare_dram_parameter("cb", [P, N], dt, isOutput=False)
    out = nc.declare_dram_parameter("out", [MS, N], dt, isOutput=True)

    from contextlib import ExitStack

    with ExitStack() as ctx:
        X = ctx.enter_context(nc.sbuf_tensor([P, NB * N], dt))
        E = ctx.enter_context(nc.sbuf_tensor([P, NB * N], dt))
        O = ctx.enter_context(nc.sbuf_tensor([P, NB * N], dt))
        S = ctx.enter_context(nc.sbuf_tensor([P, NB], dt))
        CB = ctx.enter_context(nc.sbuf_tensor([P, N], dt))
        EC = ctx.enter_context(nc.sbuf_tensor([P, NB * N], dt))
        in_s = [
            ctx.enter_context(nc.semaphore(name=f"in_s{b}")) for b in range(NB)
        ]
        out_s = [
            ctx.enter_context(nc.semaphore(name=f"out_s{b}")) for b in range(NB)
        ]
        exp_sem = ctx.enter_context(nc.semaphore(name="exp_sem"))
        ln_sem = ctx.enter_context(nc.semaphore(name="ln_sem"))
        cb_sem = ctx.enter_context(nc.semaphore(name="cb_sem"))
        mul_sem = ctx.enter_context(nc.semaphore(name="mul_sem"))
        block = ctx.enter_context(nc.Block())

        @block.sync
        def _(sync):
            sync.dma_start(out=CB[:], in_=cb[:]).then_inc(cb_sem, 16)
            for t in range(NT):
                b = t % NB
                if t >= NB:
                    sync.wait_ge(exp_sem, t - NB + 1)
                sync.dma_start(
                    out=X[:, b * N:(b + 1) * N], in_=x[t * P:(t + 1) * P, :]
                ).then_inc(in_s[b], 16)
            for b in range(NB):
                sync.wait_ge(out_s[b], (NT // NB) * 16)

        @block.vector
        def _(vector):
            vector.wait_ge(cb_sem, 16)
            for t in range(NT):
                b = t % NB
                vector.wait_ge(exp_sem, t + 1)
                if t >= NB:
                    vector.wait_ge(ln_sem, t - NB + 1)
                vector.tensor_mul(
                    EC[:, b * N:(b + 1) * N], E[:, b * N:(b + 1) * N], CB[:]
                ).then_inc(mul_sem, 1)

        @block.scalar
        def _(scalar):
            for t in range(NT):
                b, k = t % NB, t // NB
                scalar.wait_ge(in_s[b], (k + 1) * 16)
                if t >= NB:
                    scalar.wait_ge(mul_sem, t - NB + 1)
                scalar.activation(
                    E[:, b * N:(b + 1) * N],
                    X[:, b * N:(b + 1) * N],
                    AF.Exp,
                    accum_out=S[:, b:b + 1],
                ).then_inc(exp_sem, 1)
                scalar.wait_ge(mul_sem, t + 1)
                if t >= NB:
                    scalar.wait_ge(out_s[b], k * 16)
                scalar.activation(
                    O[:, b * N:(b + 1) * N],
                    EC[:, b * N:(b + 1) * N],
                    AF.Ln,
                    bias=S[:, b:b + 1],
                    scale=1.0,
                ).then_inc(ln_sem, 1)
                scalar.wait_ge(ln_sem, t + 1)
                scalar.dma_start(
                    out=out[t * P:(t + 1) * P, :], in_=O[:, b * N:(b + 1) * N]
                ).then_inc(out_s[b], 16)

    return nc


def _ensure_axon_hooks_stub():
    """bass_utils imports antenv.axon_hooks when tracing under axon; some
    images lack that module. Provide a no-hook stub so a BASS_TRACE=1
    environment degrades to 'tracing skipped' instead of ImportError."""
    import sys
    import types

    try:
        import antenv.axon_hooks  # noqa: F401
    except ImportError:
        try:
            import antenv
        except ImportError:
            return
        mod = types.ModuleType("antenv.axon_hooks")
        mod._hook = None
        mod.set_axon_ntff_profile_hook = lambda h: setattr(mod, "_hook", h)
        mod.get_axon_ntff_profile_hook = lambda: mod._hook
        sys.modules["antenv.axon_hooks"] = mod
        antenv.axon_hooks = mod


def _run(xx, diag, trace=False):
    _ensure_axon_hooks_stub()
    from concourse.bass_utils import run_bass_kernel_spmd

    xx = np.asarray(xx, dtype=np.float32)
    diag = np.asarray(diag, dtype=np.float32)
    assert xx.shape == (N, M) and diag.shape == (N,)

    ones = bool(np.all(diag == 1.0))
    key = "ones" if ones else "general"
    if key not in _program_cache:
        _program_cache[key] = (
            _build_fast(float(np.expm1(1.0))) if ones else _build_general()
        )
    nc = _program_cache[key]

    xxT = np.ascontiguousarray(xx.T)  # (M, N): row m holds all j of column m
    in_maps = []
    for c in range(NCORES):
        m = {"x": xxT[c * MS:(c + 1) * MS]}
        if not ones:
            cbv = np.ascontiguousarray(
                np.broadcast_to(np.expm1(diag)[None, :], (P, N))
            ).astype(np.float32)
            m["cb"] = cbv
        in_maps.append(m)

    res = run_bass_kernel_spmd(nc, in_maps, core_ids=list(range(NCORES)), trace=trace)
    outT = np.concatenate([r["out"] for r in res.results], axis=0)  # (M, N)
    return np.ascontiguousarray(outT.T), res


def kernel(xx, diag):
    out, _ = _run(xx, diag)
    return out
